# revision 15
# baseline (speedup 1.0000x reference)
"""Trainium2 Bass kernel for nn_DQN_9904194584789 (GNN message passing DQN).

Reference math (B=16, A=256, U=64, T=3):
    cur_sol = x[:,0,:]; mask = x[:,1,:]; w = x[:,2:,:]          # [B,A,A]
    adj = (w != 0)
    e1 = cur_sol[:,:,None] @ W0                                  # rank-1
    e3 = (sum_j relu(w[...,None] * W3) / A) @ W2                 # collapses:
         sum_j relu(w*c) = P*relu(c) + N*relu(-c) elementwise in c, with
         P = sum_j relu(w), N = P - S, S = sum_j w
         => e3 = (P/A) x (rp + rn) - (S/A) x rn,
            rp = relu(W3)@W2, rn = relu(-W3)@W2
    base = e1 + e3 (loop invariant);  emb_1 = relu(base)
    emb_{t+1} = relu(base + (adj @ emb_t / A) @ W1)
    heads: dueling MLP on emb_T, row-sum corrections, + 10*mask.

Sharding: pure data-parallel over batch B (2 batches per core x 8 cores).

Fast path (w has no exact zeros -> adj all-ones, checked on host): BOTH
batches are processed in single ops using a stacked-partition layout
(partition p = b*64+u) with host-packed block-diagonal weights, so each
step of the serial chain is one instruction instead of two.  Row sums of
w (P = sum relu, S = sum) are computed with tensor_scalar+accum_out
(2x DVE mode on f32 SBUF), pipelined against the 4 chunked w DMAs.
Per-iteration message passing is a single f32 matmul (W1 block-diag / A,
LOW_HIGH) + one fused add-bias-relu-accum vector op reading base from
PSUM.  Heads are block-diagonal matmuls; the row-sum of the advantage
head is folded into a tiny extra matmul on the h2 accumulator instead of
a vector reduce.

General path (exact zeros present) is the faithful per-batch adjacency
version, unchanged from the previous revision.

Precision: fp16 operands for all matmuls except the per-iteration c
matmul and the tiny v/ra dots (f32, PE LOW_HIGH mode).  PSUM
accumulation is always f32.  ~5e-4 scale-relative error.
"""

import numpy as np
from contextlib import ExitStack

import concourse.bass as bass
import concourse.bacc as bacc
import concourse.tile as tile
from concourse import mybir
from concourse.bass_utils import run_bass_kernel_spmd
from concourse.masks import make_identity

f32 = mybir.dt.float32
f16 = mybir.dt.float16
Alu = mybir.AluOpType
Act = mybir.ActivationFunctionType
AxX = mybir.AxisListType.X

B, A, U, HID = 16, 256, 64, 64
NCORES = 8
BPC = B // NCORES  # batches per core
INV_A = 1.0 / A

WEIGHT_NAMES = [
    "W0", "W1", "W2", "W3",
    "a_w1", "a_b1", "a_w2", "a_b2", "a_w3", "a_b3",
    "v_w1", "v_b1", "v_w2", "v_b2",
]


def _place_f16(wp: np.ndarray, row0: int, col: int, arr: np.ndarray):
    """Pack fp16 bits of arr pairwise into f32 columns of wp."""
    raw = np.ascontiguousarray(
        np.asarray(arr, np.float32).astype(np.float16)
    ).view(np.uint16)
    k = raw.shape[1]
    pad = np.zeros((raw.shape[0], (k + 1) // 2 * 2), np.uint16)
    pad[:, :k] = raw
    fview = pad.view(np.float32)
    wp[row0:row0 + fview.shape[0], col:col + fview.shape[1]] = fview


# ---------------------------------------------------------------------------
# FAST PATH (no exact zeros in w): merged-batch stacked-partition kernel
# ---------------------------------------------------------------------------
# wp2 [128, NW2] f32 column layout (host-packed, one DMA):
FW_W1H = 0     # fp16 hi(block_diag(W1, W1)/A)       [128, 128] -> 64 cols
FW_W1L = 64    # fp16 lo residual of same            [128, 128] -> 64 cols
FW_F2 = 128    # fp16 F2 stationary                  [6, 128] -> 64 cols
FW_AW1 = 192   # fp16 block_diag(a_w1)               [128, 128] -> 64 cols
FW_VW1 = 256   # fp16 block_diag(v_w1)               [128, 128] -> 64 cols
FW_AW2 = 320   # fp16 block_diag(a_w2)               [128, 64] -> 32 cols
FW_AW3 = 352   # fp16 stacked a_w3                   [64, 2] -> 1 col
FW_VW2 = 353   # f32 stacked v_w2                    [128, 2]
FW_AB1 = 355   # f32 [a_b1; a_b1]                    [128, 1]
FW_VB1 = 356   # f32 [v_b1; v_b1]                    [128, 1]
FW_AB2 = 357   # f32 [a_b2; a_b2]                    [64, 1]
FW_KC = 358    # f32 kc = A*v_b2 - (A-1)*a_b3        [2, 1]
FW_AW3F = 359  # f32 stacked -a_w3 (negated for psv-ra accum) [64, 2]
NW2 = 362


def _pack_weights_fast(inputs: dict) -> np.ndarray:
    wp = np.zeros((128, NW2), dtype=np.float32)

    def bd(m):
        z = np.zeros_like(m)
        return np.block([[m, z], [z, m]])

    W1 = np.asarray(inputs["W1"], np.float32)
    w1a = bd(W1) * INV_A
    w1h = w1a.astype(np.float16).astype(np.float32)
    _place_f16(wp, 0, FW_W1H, w1a)
    _place_f16(wp, 0, FW_W1L, w1a - w1h)

    W3 = np.asarray(inputs["W3"], np.float32)[0]          # [64]
    W2m = np.asarray(inputs["W2"], np.float32)            # [64, 64]
    rp = np.maximum(W3, 0.0) @ W2m                        # [64]
    rn = np.maximum(-W3, 0.0) @ W2m
    w0 = np.asarray(inputs["W0"], np.float32)[0]          # [64]
    # G2 rows: [P_b0, S_b0, P_b1, S_b1, cs_b0, cs_b1]
    F2 = np.zeros((6, 128), np.float32)
    F2[0, 0:64] = (rp + rn) * INV_A
    F2[1, 0:64] = -rn * INV_A
    F2[2, 64:128] = (rp + rn) * INV_A
    F2[3, 64:128] = -rn * INV_A
    F2[4, 0:64] = w0
    F2[5, 64:128] = w0
    _place_f16(wp, 0, FW_F2, F2)

    _place_f16(wp, 0, FW_AW1, bd(np.asarray(inputs["a_w1"], np.float32)))
    _place_f16(wp, 0, FW_VW1, bd(np.asarray(inputs["v_w1"], np.float32)))
    _place_f16(wp, 0, FW_AW2, bd(np.asarray(inputs["a_w2"], np.float32)))
    aw3 = np.asarray(inputs["a_w3"], np.float32)[:, 0]    # [32]
    A3 = np.zeros((64, 2), np.float32)
    A3[0:32, 0] = aw3
    A3[32:64, 1] = aw3
    _place_f16(wp, 0, FW_AW3, A3)
    v2 = np.asarray(inputs["v_w2"], np.float32)[:, 0]     # [64]
    wp[0:64, FW_VW2] = v2
    wp[64:128, FW_VW2 + 1] = v2
    ab1 = np.asarray(inputs["a_b1"], np.float32)
    vb1 = np.asarray(inputs["v_b1"], np.float32)
    ab2 = np.asarray(inputs["a_b2"], np.float32)
    wp[:, FW_AB1] = np.concatenate([ab1, ab1])
    wp[:, FW_VB1] = np.concatenate([vb1, vb1])
    wp[0:64, FW_AB2] = np.concatenate([ab2, ab2])
    kc = float(A) * float(np.asarray(inputs["v_b2"])[0]) \
        - float(A - 1) * float(np.asarray(inputs["a_b3"])[0])
    wp[0:2, FW_KC] = kc
    wp[0:64, FW_AW3F:FW_AW3F + 2] = -A3
    return wp


def _build_fast() -> bass.Bass:
    nc = bacc.Bacc(
        "TRN2", target_bir_lowering=False, debug=False, num_devices=NCORES
    )
    xs = nc.declare_dram_parameter("xs", [BPC, A + 2, A], f32, isOutput=False)
    wpd = nc.declare_dram_parameter("wp2", [128, NW2], f32, isOutput=False)
    out = nc.declare_dram_parameter("out", [BPC, A], f32, isOutput=True)

    with tile.TileContext(nc) as tc, ExitStack() as ctx:
        cp = ctx.enter_context(tc.tile_pool(name="const", bufs=1))
        sp = ctx.enter_context(tc.tile_pool(name="scratch", bufs=2))

        # ---------- input DMAs, earliest + spread across queues -----------
        # w tiles chunked (b, t) so reductions pipeline with arrival.
        wt4 = cp.tile([128, 4, 256], f32, tag="wt4")
        wp = cp.tile([128, NW2], f32, tag="wp")
        dma_order = [(0, 0, nc.sync), (1, 0, nc.scalar),
                     (0, 1, nc.sync), (1, 1, nc.gpsimd)]
        def wdma(b, t, eng):
            eng.dma_start(
                wt4[:, 2 * b + t, :],
                xs[b, 2 + t * 128: 2 + (t + 1) * 128, :],
            )
        wdma(0, 0, nc.sync)
        wdma(1, 0, nc.scalar)
        wdma(1, 1, nc.gpsimd)
        wdma(0, 1, nc.sync)
        # replicated packed weights on the gpsimd (SWDGE) queue
        nc.gpsimd.dma_start(wp[:], wpd[:])

        # G2 moving tensor [6, 256] fp16; rows 4:6 = cur_sol, cast in-DMA
        G2 = cp.tile([6, 256], f16, tag="G2")
        nc.gpsimd.dma_start(G2[4:6, :], xs[:, 0, :])
        # mask rows [2, 256] f32
        mrowf = cp.tile([2, 256], f32, tag="mrowf")
        nc.scalar.dma_start(mrowf[:], xs[:, 1, :])

        # fp16 weight views
        F2v = wp[0:6, FW_F2:FW_F2 + 64].bitcast(f16)        # [6, 128]
        aw1v = wp[:, FW_AW1:FW_AW1 + 64].bitcast(f16)       # [128, 128]
        vw1v = wp[:, FW_VW1:FW_VW1 + 64].bitcast(f16)       # [128, 128]
        aw2v = wp[:, FW_AW2:FW_AW2 + 32].bitcast(f16)       # [128, 64]
        aw3v = wp[0:64, FW_AW3:FW_AW3 + 1].bitcast(f16)     # [64, 2]
        vw2v = wp[:, FW_VW2:FW_VW2 + 2]                     # [128, 2] f32
        w1hv = wp[:, FW_W1H:FW_W1H + 64].bitcast(f16)       # [128, 128]
        w1lv = wp[:, FW_W1L:FW_W1L + 64].bitcast(f16)       # [128, 128]

        ident = cp.tile([128, 128], f16, tag="ident")
        make_identity(nc, ident[:])
        zeros = cp.tile([128, 256], f32, tag="zeros")
        nc.gpsimd.memset(zeros[:], 0.0)

        # m10k = mask * 10 + kc (early, off the critical chain)
        m10f = cp.tile([2, 256], f32, tag="m10f")
        nc.vector.tensor_scalar(
            m10f[:], mrowf[:], 10.0, wp[0:2, FW_KC:FW_KC + 1],
            Alu.mult, op1=Alu.add,
        )

        with tc.tile_pool(name="pg", bufs=1, space="PSUM") as pg, \
             tc.tile_pool(name="pb", bufs=1, space="PSUM") as pb, \
             tc.tile_pool(name="pc", bufs=1, space="PSUM") as pc, \
             tc.tile_pool(name="ph", bufs=2, space="PSUM") as ph, \
             tc.tile_pool(name="pf", bufs=1, space="PSUM") as pf:

            # ---- P/S row sums via TS+accum, one op per (b, t) tile ------
            # Gc cols (t-major): [P_b0, S_b0, P_b1, S_b1] per t
            Gc = cp.tile([128, 8], f16, tag="Gc")
            for t, border in ((0, (0, 1)), (1, (1, 0))):
                for b in border:
                    wtile = wt4[:, 2 * b + t, :]
                    dS = sp.tile([128, 256], f16, tag="dump")
                    nc.vector.tensor_scalar(
                        dS[:], wtile, 0.0, None, Alu.add, op1=Alu.add,
                        accum_out=Gc[:, t * 4 + 2 * b + 1: t * 4 + 2 * b + 2],
                    )
                    dP = sp.tile([128, 256], f16, tag="dump")
                    nc.vector.tensor_scalar(
                        dP[:], wtile, 0.0, None, Alu.max, op1=Alu.add,
                        accum_out=Gc[:, t * 4 + 2 * b: t * 4 + 2 * b + 1],
                    )
                # transpose this t's 4 columns as soon as they are ready
                psT = pg.tile([4, 128], f16, tag="psT")
                nc.tensor.transpose(
                    psT[:], Gc[:, t * 4:(t + 1) * 4], ident[:]
                )
                nc.vector.tensor_copy(G2[0:4, t * 128:(t + 1) * 128], psT[:])

            # ---- base [128, 256] = F2.T @ G2 (K=6 fp16 matmul) ----------
            ps_base = pb.tile([128, 256], f32, tag="psbase")
            nc.tensor.matmul(ps_base[:], F2v, G2[:])
            base_sb = cp.tile([128, 256], f32, tag="base_sb")
            nc.scalar.copy(base_sb[:], ps_base[:])

            # ---- message passing: 2 bias updates off colsums ------------
            cs1 = sp.tile([128, 1], f16, tag="cs1")
            d0 = sp.tile([128, 256], f16, tag="dump")
            nc.vector.tensor_scalar(
                d0[:], ps_base[:], 0.0, None, Alu.max, op1=Alu.add,
                accum_out=cs1[:],
            )
            ps_c1 = pc.tile([128, 1], f32, tag="psc")
            nc.tensor.matmul(ps_c1[:], w1hv, cs1[:], start=True, stop=False)
            nc.tensor.matmul(ps_c1[:], w1lv, cs1[:], start=False, stop=True)

            cs2 = sp.tile([128, 1], f16, tag="cs2")
            d1 = sp.tile([128, 256], f16, tag="dump")
            nc.vector.scalar_tensor_tensor(
                d1[:], base_sb[:], ps_c1[:, 0:1], zeros[:],
                Alu.add, Alu.max, accum_out=cs2[:],
            )
            ps_c2 = pc.tile([128, 1], f32, tag="psc")
            nc.tensor.matmul(ps_c2[:], w1hv, cs2[:], start=True, stop=False)
            nc.tensor.matmul(ps_c2[:], w1lv, cs2[:], start=False, stop=True)

            EMB = sp.tile([128, 256], f16, tag="EMB")
            nc.vector.tensor_scalar(
                EMB[:], base_sb[:], ps_c2[:, 0:1], 0.0, Alu.add, op1=Alu.max
            )

            # ---- dueling heads (block-diagonal, both batches at once) ---
            ph1 = ph.tile([128, 256], f32, tag="pmat")
            nc.tensor.matmul(ph1[:], aw1v, EMB[:])
            h1 = sp.tile([128, 256], f16, tag="h1")
            nc.scalar.activation(h1[:], ph1[:], Act.Relu,
                                 bias=wp[:, FW_AB1:FW_AB1 + 1])

            phv = ph.tile([128, 256], f32, tag="pmat")
            nc.tensor.matmul(phv[:], vw1v, EMB[:])
            hvd = sp.tile([128, 256], f16, tag="hvd")
            hvcs = sp.tile([128, 1], f32, tag="hvcs")
            nc.scalar.activation(
                hvd[:], phv[:], Act.Relu, bias=wp[:, FW_VB1:FW_VB1 + 1],
                accum_out=hvcs[:],
            )
            pkb = pf.tile([2, 1], f32, tag="pkb")
            nc.tensor.matmul(pkb[:], vw2v, hvcs[:], start=True, stop=False)

            ph2 = ph.tile([64, 256], f32, tag="pmat")
            nc.tensor.matmul(ph2[:], aw2v, h1[:])
            h2 = sp.tile([64, 256], f16, tag="h2")
            h2cs = sp.tile([64, 1], f32, tag="h2cs")
            nc.vector.scalar_tensor_tensor(
                h2[:], ph2[:], wp[0:64, FW_AB2:FW_AB2 + 1], zeros[0:64, :],
                Alu.add, Alu.max, accum_out=h2cs[:],
            )
            pa = pf.tile([2, 256], f32, tag="pa")
            nc.tensor.matmul(pa[:], aw3v, h2[:])
            # pkb += (-a_w3bd)^T @ h2cs  ->  pkb = psv - ra
            nc.tensor.matmul(pkb[:], wp[0:64, FW_AW3F:FW_AW3F + 2], h2cs[:],
                             start=False, stop=True)

            # out = pa + [psv - ra] + (10*mask + kc)
            FIN = cp.tile([2, 256], f32, tag="FIN")
            nc.vector.scalar_tensor_tensor(
                FIN[:], pa[:], pkb[:, 0:1], m10f[:], Alu.add, Alu.add
            )
            nc.sync.dma_start(out[:, :], FIN[:])

    return nc


# ---------------------------------------------------------------------------
# GENERAL PATH (exact zeros in w): faithful per-batch adjacency matmuls
# (unchanged baseline implementation)
# ---------------------------------------------------------------------------
# wpack [64, NWP] f32 column layout (host-packed replicated params, one DMA).
WP_W1 = 0          # [64, 64] f32
WP_W2 = 64         # [64, 64] f32
WP_W3 = 128        # [64, 1] f32 column
WP_AB1 = 129       # [64, 1] f32
WP_VB1 = 130       # [64, 1] f32
WP_AB2 = 131       # [32, 1] f32 (padded)
WP_VW2 = 132       # [64, 1] f32
WP_AB3 = 133       # scalar at [0, 133]
WP_VB2 = 134       # scalar at [0, 134]
WP_AW1H = 135      # [64, 64] fp16 -> 32 f32 cols
WP_AW2H = 167      # [64, 32] fp16 -> 16 f32 cols
WP_AW3H = 183      # [32, 1] fp16 padded -> 1 f32 col
WP_VW1H = 184      # [64, 64] fp16 -> 32 f32 cols
WP_W0C = 216       # [64, 1] fp16 column (W0 transposed) -> 1 f32 col
WP_W2H = 217       # [64, 64] fp16 -> 32 f32 cols
NWP = 249


def _pack_weights(inputs: dict) -> np.ndarray:
    wp = np.zeros((64, NWP), dtype=np.float32)
    wp[:, WP_W1:WP_W1 + 64] = inputs["W1"]
    wp[:, WP_W2:WP_W2 + 64] = inputs["W2"]
    wp[:, WP_W3] = inputs["W3"][0]
    wp[:, WP_AB1] = inputs["a_b1"]
    wp[:, WP_VB1] = inputs["v_b1"]
    wp[:32, WP_AB2] = inputs["a_b2"]
    wp[:, WP_VW2] = inputs["v_w2"][:, 0]
    wp[0, WP_AB3] = inputs["a_b3"][0]
    wp[0, WP_VB2] = inputs["v_b2"][0]

    _place_f16(wp, 0, WP_AW1H, inputs["a_w1"])
    _place_f16(wp, 0, WP_AW2H, inputs["a_w2"])
    _place_f16(wp, 0, WP_AW3H, np.asarray(inputs["a_w3"])[:, 0:1])
    _place_f16(wp, 0, WP_VW1H, inputs["v_w1"])
    _place_f16(wp, 0, WP_W0C, np.asarray(inputs["W0"]).T)   # [64, 1]
    _place_f16(wp, 0, WP_W2H, inputs["W2"])
    return wp


def _build_general() -> bass.Bass:
    nc = bacc.Bacc(
        "TRN2", target_bir_lowering=False, debug=False, num_devices=NCORES
    )
    xs = nc.declare_dram_parameter("xs", [BPC, A + 2, A], f32, isOutput=False)
    wpd = nc.declare_dram_parameter("wpack", [64, NWP], f32, isOutput=False)
    out = nc.declare_dram_parameter("out", [BPC, A], f32, isOutput=True)

    with tile.TileContext(nc) as tc, ExitStack() as ctx:
        cp = ctx.enter_context(tc.tile_pool(name="const", bufs=1))
        sp = ctx.enter_context(tc.tile_pool(name="scratch", bufs=2))

        wp = cp.tile([64, NWP], f32, tag="wp")
        nc.sync.dma_start(wp[:], wpd[:])
        wt4 = cp.tile([128, 2 * BPC, A], f32, tag="wt4")
        for b in range(BPC):
            nc.scalar.dma_start(
                wt4[:, 2 * b: 2 * b + 2, :],
                xs[b, 2: A + 2, :].rearrange("(t p) j -> p t j", p=128),
            )
        csc = cp.tile([128, 2 * BPC], f32, tag="csc")
        for b in range(BPC):
            nc.gpsimd.dma_start(
                csc[:, 2 * b: 2 * b + 2],
                xs[b, 0, :].rearrange("(t p) -> p t", p=128),
            )
        mrow = cp.tile([1, BPC * A], f32, tag="mrow")
        nc.gpsimd.dma_start(
            mrow[:].rearrange("p (b a) -> p b a", b=BPC),
            xs[:, 1, :][None, :, :],
        )

        aw1h = wp[:, WP_AW1H:WP_AW1H + 32].bitcast(f16)
        aw2h = wp[:, WP_AW2H:WP_AW2H + 16].bitcast(f16)
        aw3h = wp[0:32, WP_AW3H:WP_AW3H + 1].bitcast(f16)[:, 0:1]
        vw1h = wp[:, WP_VW1H:WP_VW1H + 32].bitcast(f16)
        w0c = wp[:, WP_W0C:WP_W0C + 1].bitcast(f16)[:, 0:1]

        ident = cp.tile([128, 128], f16, tag="ident")
        make_identity(nc, ident[:])
        identf = cp.tile([128, 128], f32, tag="identf")
        make_identity(nc, identf[:])

        with tc.tile_pool(name="psetup", bufs=2, space="PSUM") as psetup:
            w2h = wp[:, WP_W2H:WP_W2H + 32].bitcast(f16)
            w3p = cp.tile([U, 1], f16, tag="w3p")
            nc.scalar.activation(w3p[:], wp[:, WP_W3:WP_W3 + 1], Act.Relu)
            w3n = cp.tile([U, 1], f16, tag="w3n")
            nc.scalar.activation(w3n[:], wp[:, WP_W3:WP_W3 + 1], Act.Relu,
                                 scale=-1.0)
            Fc = cp.tile([U, 3], f16, tag="Fc")
            nc.vector.tensor_copy(Fc[:, 0:1], w0c)
            pspc = psetup.tile([U, 1], f32, tag="pscol")
            nc.tensor.matmul(pspc[:], w2h, w3p[:])
            nc.scalar.mul(Fc[:, 1:2], pspc[:], INV_A)
            psnc = psetup.tile([U, 1], f32, tag="pscol")
            nc.tensor.matmul(psnc[:], w2h, w3n[:])
            nc.scalar.mul(Fc[:, 2:3], psnc[:], INV_A)
            psF = psetup.tile([3, U], f16, tag="psF")
            nc.tensor.transpose(psF[:], Fc[:], ident[0:U, 0:U])
            F = cp.tile([3, U], f16, tag="F")
            nc.vector.tensor_copy(F[:], psF[:])

        t256 = cp.tile([1, 1], f32, tag="t256")
        nc.gpsimd.tensor_scalar(
            t256[:], wp[0:1, WP_VB2:WP_VB2 + 1], float(A), None, Alu.mult
        )
        kc = cp.tile([1, 1], f32, tag="kc")
        nc.gpsimd.tensor_scalar(
            kc[:], wp[0:1, WP_AB3:WP_AB3 + 1], -float(A - 1), t256[:],
            Alu.mult, Alu.add,
        )

        m10 = cp.tile([1, BPC * A], f32, tag="m10")
        nc.scalar.mul(m10[:], mrow[:], 10.0)

        FIN = cp.tile([1, BPC * A], f32, tag="FIN")

        with tc.tile_pool(name="pmm", bufs=1, space="PSUM") as pmm, \
             tc.tile_pool(name="pbase", bufs=2, space="PSUM") as pbase, \
             tc.tile_pool(name="phead", bufs=2, space="PSUM") as phead:
            for b in range(BPC):
                Tb = sp.tile([128, 2], f32, tag="Tb")
                nc.vector.tensor_reduce(
                    Tb[:], wt4[:, 2 * b: 2 * b + 2, :], axis=AxX, op=Alu.add,
                    apply_absolute_value=True,
                )
                Sb = sp.tile([128, 2], f32, tag="Sb")
                nc.vector.tensor_reduce(
                    Sb[:], wt4[:, 2 * b: 2 * b + 2, :], axis=AxX, op=Alu.add
                )
                Sh = sp.tile([128, 2], f32, tag="Sh")
                nc.gpsimd.tensor_scalar(Sh[:], Sb[:], 0.5, None, Alu.mult)

                G = sp.tile([3, A], f16, tag="G")
                for t in range(2):
                    Cc = sp.tile([128, 3], f16, tag="Cc")
                    nc.gpsimd.tensor_copy(
                        Cc[:, 0:1], csc[:, 2 * b + t: 2 * b + t + 1]
                    )
                    nc.vector.scalar_tensor_tensor(
                        Cc[:, 1:2], Tb[:, t: t + 1], 0.5, Sh[:, t: t + 1],
                        Alu.mult, Alu.add,
                    )
                    nc.vector.scalar_tensor_tensor(
                        Cc[:, 2:3], Tb[:, t: t + 1], 0.5, Sh[:, t: t + 1],
                        Alu.mult, Alu.subtract,
                    )
                    tpc = pmm.tile([3, 128], f16, tag="tp1")
                    nc.tensor.transpose(tpc[:], Cc[:], ident[:])
                    nc.vector.tensor_copy(
                        G[:, t * 128: (t + 1) * 128], tpc[:]
                    )

                ps_base = pbase.tile([U, A], f32, tag="psbase")
                nc.tensor.matmul(ps_base[:], F[:], G[:])

                wt = wt4[:, 2 * b: 2 * b + 2, :]
                adjT = sp.tile([128, 2, A], f32, tag="adjT")
                for at in range(2):
                    for jt in range(2):
                        ptr = pmm.tile([128, 128], f32, tag="tp1")
                        nc.tensor.transpose(
                            ptr[:], wt[:, at, jt * 128: (jt + 1) * 128],
                            identf[:],
                        )
                        nc.vector.tensor_scalar(
                            adjT[:, jt, at * 128: (at + 1) * 128],
                            ptr[:], 0.0, None, Alu.not_equal,
                        )
                embT = sp.tile([U, A], f32, tag="embT")
                nc.vector.tensor_scalar(
                    embT[:], ps_base[:], 0.0, None, Alu.max
                )
                EMBb = None
                for it in range(2):
                    nat = sp.tile([128, 2, U], f32, tag="nat")
                    for ht in range(2):
                        pnat = pmm.tile([128, U], f32, tag="tp1")
                        nc.tensor.transpose(
                            pnat[:], embT[:, ht * 128: (ht + 1) * 128],
                            identf[0:U, 0:U],
                        )
                        nc.vector.tensor_copy(nat[:, ht, :], pnat[:])
                    ps_y = pmm.tile([U, A], f32, tag="tp1")
                    nc.tensor.matmul(ps_y[:], nat[:, 0, :], adjT[:, 0, :],
                                     start=True, stop=False)
                    nc.tensor.matmul(ps_y[:], nat[:, 1, :], adjT[:, 1, :],
                                     start=False, stop=True)
                    ysb = sp.tile([U, A], f32, tag="ysb")
                    nc.vector.tensor_scalar(ysb[:], ps_y[:], INV_A, None,
                                            Alu.mult)
                    ps_it = pbase.tile([U, A], f32, tag="psbase")
                    nc.tensor.matmul(ps_it[:], F[:], G[:],
                                     start=True, stop=False)
                    nc.tensor.matmul(ps_it[:], wp[:, WP_W1:WP_W1 + 64],
                                     ysb[:], start=False, stop=True)
                    if it == 0:
                        embT = sp.tile([U, A], f32, tag="embT")
                        nc.vector.tensor_scalar(
                            embT[:], ps_it[:], 0.0, None, Alu.max
                        )
                    else:
                        EMBb = sp.tile([U, A], f16, tag="EMBb")
                        nc.vector.tensor_scalar(
                            EMBb[:], ps_it[:], 0.0, None, Alu.max
                        )

                sl = slice(b * A, (b + 1) * A)
                ph1 = phead.tile([HID, A], f32, tag="pmat")
                nc.tensor.matmul(ph1[:], aw1h, EMBb[:])
                h1 = sp.tile([HID, A], f16, tag="h1")
                nc.scalar.activation(h1[:], ph1[:], Act.Relu,
                                     bias=wp[:, WP_AB1:WP_AB1 + 1])
                ph2 = phead.tile([HID // 2, A], f32, tag="pmat")
                nc.tensor.matmul(ph2[:], aw2h, h1[:])
                h2 = sp.tile([HID // 2, A], f16, tag="h2")
                nc.vector.tensor_scalar(
                    h2[:], ph2[:], wp[0:32, WP_AB2:WP_AB2 + 1], 0.0,
                    Alu.add, op1=Alu.max,
                )
                pa = phead.tile([1, A], f32, tag="pa")
                nc.tensor.matmul(pa[:], aw3h, h2[:])

                phv = phead.tile([HID, A], f32, tag="pmat")
                nc.tensor.matmul(phv[:], vw1h, EMBb[:])
                hv = sp.tile([HID, A], f32, tag="hv")
                hv_cs = sp.tile([U, 1], f32, tag="hv_cs")
                nc.scalar.activation(hv[:], phv[:], Act.Relu,
                                     bias=wp[:, WP_VB1:WP_VB1 + 1],
                                     accum_out=hv_cs[:])
                psv = phead.tile([1, 1], f32, tag="pa")
                nc.tensor.matmul(psv[:], hv_cs[:], wp[:, WP_VW2:WP_VW2 + 1])

                ra = sp.tile([1, 1], f32, tag="ra")
                nc.vector.tensor_reduce(ra[:], pa[:], axis=AxX, op=Alu.add)
                Kb = sp.tile([1, 1], f32, tag="Kb")
                nc.vector.tensor_scalar(
                    Kb[:], psv[:], ra[:], kc[:], Alu.subtract, op1=Alu.add
                )
                nc.vector.scalar_tensor_tensor(
                    FIN[:, sl], pa[:], Kb[:], m10[:, sl], Alu.add, Alu.add
                )
                if b == 0:
                    nc.sync.dma_start(out[b, :][None, :], FIN[:, sl])
                else:
                    nc.scalar.dma_start(out[b, :][None, :], FIN[:, sl])

    return nc


_NC_CACHE: dict[bool, bass.Bass] = {}


def _get_nc(fast: bool) -> bass.Bass:
    if fast not in _NC_CACHE:
        nc = _build_fast() if fast else _build_general()
        nc.finalize()
        _NC_CACHE[fast] = nc
    return _NC_CACHE[fast]


def _make_in_maps(inputs: dict, fast: bool) -> list[dict]:
    x = np.ascontiguousarray(np.asarray(inputs["x"], dtype=np.float32))
    wd = {k: np.asarray(inputs[k], dtype=np.float32) for k in WEIGHT_NAMES}
    wname = "wp2" if fast else "wpack"
    wpk = _pack_weights_fast(wd) if fast else _pack_weights(wd)
    in_maps = []
    for c in range(NCORES):
        in_maps.append({
            "xs": np.ascontiguousarray(x[c * BPC: (c + 1) * BPC]),
            wname: wpk,
        })
    return in_maps


def run(inputs: dict, trace: bool = False, tmpdir: str | None = None):
    """Returns (output [B, A] f32, BassKernelResults)."""
    x = np.asarray(inputs["x"])
    fast = bool((x[:, 2:, :] != 0.0).all())
    nc = _get_nc(fast)
    res = run_bass_kernel_spmd(
        nc, _make_in_maps(inputs, fast), list(range(NCORES)),
        trace=trace, tmpdir=tmpdir,
    )
    out = np.concatenate([res.results[i]["out"] for i in range(NCORES)], axis=0)
    return out, res


def kernel(**inputs) -> np.ndarray:
    out, _ = run(inputs)
    return out


# revision 16
# speedup vs baseline: 1.0626x; 1.0626x over previous
"""Trainium2 Bass kernel for nn_DQN_9904194584789 (GNN message passing DQN).

Reference math (B=16, A=256, U=64, T=3):
    cur_sol = x[:,0,:]; mask = x[:,1,:]; w = x[:,2:,:]          # [B,A,A]
    adj = (w != 0)
    e1 = cur_sol[:,:,None] @ W0                                  # rank-1
    e3 = (sum_j relu(w[...,None] * W3) / A) @ W2                 # collapses:
         sum_j relu(w*c) = P*relu(c) + N*relu(-c) elementwise in c, with
         P = sum_j relu(w), N = P - S, S = sum_j w
         => e3 = (P/A) x (rp + rn) - (S/A) x rn,
            rp = relu(W3)@W2, rn = relu(-W3)@W2
    base = e1 + e3 (loop invariant);  emb_1 = relu(base)
    emb_{t+1} = relu(base + (adj @ emb_t / A) @ W1)
    heads: dueling MLP on emb_T, row-sum corrections, + 10*mask.

Sharding: pure data-parallel over batch B (2 batches per core x 8 cores).

Fast path (w has no exact zeros -> adj all-ones, checked on host): BOTH
batches are processed in single ops using a stacked-partition layout
(partition p = b*64+u) with host-packed block-diagonal weights, so each
step of the serial chain is one instruction instead of two.  Row sums of
w (P = sum relu, S = sum) are computed with tensor_scalar+accum_out
(2x DVE mode on f32 SBUF), pipelined against the 4 chunked w DMAs.
Per-iteration message passing is a single f32 matmul (W1 block-diag / A,
LOW_HIGH) + one fused add-bias-relu-accum vector op reading base from
PSUM.  Heads are block-diagonal matmuls; the row-sum of the advantage
head is folded into a tiny extra matmul on the h2 accumulator instead of
a vector reduce.

General path (exact zeros present) is the faithful per-batch adjacency
version, unchanged from the previous revision.

Precision: fp16 operands for all matmuls except the per-iteration c
matmul and the tiny v/ra dots (f32, PE LOW_HIGH mode).  PSUM
accumulation is always f32.  ~5e-4 scale-relative error.
"""

import numpy as np
from contextlib import ExitStack

import concourse.bass as bass
import concourse.bacc as bacc
import concourse.tile as tile
from concourse import mybir
from concourse.bass_utils import run_bass_kernel_spmd
from concourse.masks import make_identity

f32 = mybir.dt.float32
f16 = mybir.dt.float16
Alu = mybir.AluOpType
Act = mybir.ActivationFunctionType
AxX = mybir.AxisListType.X

B, A, U, HID = 16, 256, 64, 64
NCORES = 8
BPC = B // NCORES  # batches per core
INV_A = 1.0 / A

WEIGHT_NAMES = [
    "W0", "W1", "W2", "W3",
    "a_w1", "a_b1", "a_w2", "a_b2", "a_w3", "a_b3",
    "v_w1", "v_b1", "v_w2", "v_b2",
]


def _place_f16(wp: np.ndarray, row0: int, col: int, arr: np.ndarray):
    """Pack fp16 bits of arr pairwise into f32 columns of wp."""
    raw = np.ascontiguousarray(
        np.asarray(arr, np.float32).astype(np.float16)
    ).view(np.uint16)
    k = raw.shape[1]
    pad = np.zeros((raw.shape[0], (k + 1) // 2 * 2), np.uint16)
    pad[:, :k] = raw
    fview = pad.view(np.float32)
    wp[row0:row0 + fview.shape[0], col:col + fview.shape[1]] = fview


# ---------------------------------------------------------------------------
# FAST PATH (no exact zeros in w): merged-batch stacked-partition kernel
# ---------------------------------------------------------------------------
# wp2 [128, NW2] f32 column layout (host-packed, one DMA):
FW_W1H = 0     # fp16 hi(block_diag(W1, W1)/A)       [128, 128] -> 64 cols
FW_W1L = 64    # fp16 lo residual of same            [128, 128] -> 64 cols
FW_F2 = 128    # fp16 F2 stationary                  [6, 128] -> 64 cols
FW_AW1 = 192   # fp16 block_diag(a_w1)               [128, 128] -> 64 cols
FW_VW1 = 256   # fp16 block_diag(v_w1)               [128, 128] -> 64 cols
FW_AW2 = 320   # fp16 block_diag(a_w2)               [128, 64] -> 32 cols
FW_AW3 = 352   # fp16 stacked a_w3                   [64, 2] -> 1 col
FW_VW2 = 353   # f32 stacked v_w2                    [128, 2]
FW_AB1 = 355   # f32 [a_b1; a_b1]                    [128, 1]
FW_VB1 = 356   # f32 [v_b1; v_b1]                    [128, 1]
FW_AB2 = 357   # f32 [a_b2; a_b2]                    [64, 1]
FW_KC = 358    # f32 kc = A*v_b2 - (A-1)*a_b3        [2, 1]
FW_AW3F = 359  # f32 stacked -a_w3 (negated for psv-ra accum) [64, 2]
NW2 = 362


def _pack_weights_fast(inputs: dict) -> np.ndarray:
    wp = np.zeros((128, NW2), dtype=np.float32)

    def bd(m):
        z = np.zeros_like(m)
        return np.block([[m, z], [z, m]])

    W1 = np.asarray(inputs["W1"], np.float32)
    w1a = bd(W1) * INV_A
    w1h = w1a.astype(np.float16).astype(np.float32)
    _place_f16(wp, 0, FW_W1H, w1a)
    _place_f16(wp, 0, FW_W1L, w1a - w1h)

    W3 = np.asarray(inputs["W3"], np.float32)[0]          # [64]
    W2m = np.asarray(inputs["W2"], np.float32)            # [64, 64]
    rp = np.maximum(W3, 0.0) @ W2m                        # [64]
    rn = np.maximum(-W3, 0.0) @ W2m
    w0 = np.asarray(inputs["W0"], np.float32)[0]          # [64]
    # G2 rows: [P_b0, S_b0, P_b1, S_b1, cs_b0, cs_b1]
    F2 = np.zeros((6, 128), np.float32)
    F2[0, 0:64] = (rp + rn) * INV_A
    F2[1, 0:64] = -rn * INV_A
    F2[2, 64:128] = (rp + rn) * INV_A
    F2[3, 64:128] = -rn * INV_A
    F2[4, 0:64] = w0
    F2[5, 64:128] = w0
    _place_f16(wp, 0, FW_F2, F2)

    _place_f16(wp, 0, FW_AW1, bd(np.asarray(inputs["a_w1"], np.float32)))
    _place_f16(wp, 0, FW_VW1, bd(np.asarray(inputs["v_w1"], np.float32)))
    _place_f16(wp, 0, FW_AW2, bd(np.asarray(inputs["a_w2"], np.float32)))
    aw3 = np.asarray(inputs["a_w3"], np.float32)[:, 0]    # [32]
    A3 = np.zeros((64, 2), np.float32)
    A3[0:32, 0] = aw3
    A3[32:64, 1] = aw3
    _place_f16(wp, 0, FW_AW3, A3)
    v2 = np.asarray(inputs["v_w2"], np.float32)[:, 0]     # [64]
    wp[0:64, FW_VW2] = v2
    wp[64:128, FW_VW2 + 1] = v2
    ab1 = np.asarray(inputs["a_b1"], np.float32)
    vb1 = np.asarray(inputs["v_b1"], np.float32)
    ab2 = np.asarray(inputs["a_b2"], np.float32)
    wp[:, FW_AB1] = np.concatenate([ab1, ab1])
    wp[:, FW_VB1] = np.concatenate([vb1, vb1])
    wp[0:64, FW_AB2] = np.concatenate([ab2, ab2])
    kc = float(A) * float(np.asarray(inputs["v_b2"])[0]) \
        - float(A - 1) * float(np.asarray(inputs["a_b3"])[0])
    wp[0:2, FW_KC] = kc
    wp[0:64, FW_AW3F:FW_AW3F + 2] = -A3
    return wp


def _build_fast() -> bass.Bass:
    nc = bacc.Bacc(
        "TRN2", target_bir_lowering=False, debug=False, num_devices=NCORES
    )
    xs = nc.declare_dram_parameter("xs", [BPC, A + 2, A], f32, isOutput=False)
    wpd = nc.declare_dram_parameter("wp2", [128, NW2], f32, isOutput=False)
    out = nc.declare_dram_parameter("out", [BPC, A], f32, isOutput=True)

    with tile.TileContext(nc) as tc, ExitStack() as ctx:
        cp = ctx.enter_context(tc.tile_pool(name="const", bufs=1))
        sp = ctx.enter_context(tc.tile_pool(name="scratch", bufs=2))

        # ---------- input DMAs, earliest + spread across queues -----------
        # w tiles chunked (b, t) so reductions pipeline with arrival.
        wt4 = cp.tile([128, 4, 256], f32, tag="wt4")
        wp = cp.tile([128, NW2], f32, tag="wp")
        dma_order = [(0, 0, nc.sync), (1, 0, nc.scalar),
                     (0, 1, nc.sync), (1, 1, nc.gpsimd)]
        def wdma(b, t, eng):
            eng.dma_start(
                wt4[:, 2 * b + t, :],
                xs[b, 2 + t * 128: 2 + (t + 1) * 128, :],
            )
        wdma(0, 0, nc.sync)
        wdma(1, 0, nc.scalar)
        wdma(1, 1, nc.gpsimd)
        wdma(0, 1, nc.sync)
        # replicated packed weights on the gpsimd (SWDGE) queue
        nc.gpsimd.dma_start(wp[:], wpd[:])

        # G2 moving tensor [6, 256] fp16; rows 4:6 = cur_sol, cast in-DMA
        G2 = cp.tile([6, 256], f16, tag="G2")
        nc.gpsimd.dma_start(G2[4:6, :], xs[:, 0, :])
        # mask rows [2, 256] f32
        mrowf = cp.tile([2, 256], f32, tag="mrowf")
        nc.scalar.dma_start(mrowf[:], xs[:, 1, :])

        # fp16 weight views
        F2v = wp[0:6, FW_F2:FW_F2 + 64].bitcast(f16)        # [6, 128]
        aw1v = wp[:, FW_AW1:FW_AW1 + 64].bitcast(f16)       # [128, 128]
        vw1v = wp[:, FW_VW1:FW_VW1 + 64].bitcast(f16)       # [128, 128]
        aw2v = wp[:, FW_AW2:FW_AW2 + 32].bitcast(f16)       # [128, 64]
        aw3v = wp[0:64, FW_AW3:FW_AW3 + 1].bitcast(f16)     # [64, 2]
        vw2v = wp[:, FW_VW2:FW_VW2 + 2]                     # [128, 2] f32
        w1hv = wp[:, FW_W1H:FW_W1H + 64].bitcast(f16)       # [128, 128]
        w1lv = wp[:, FW_W1L:FW_W1L + 64].bitcast(f16)       # [128, 128]

        ident = cp.tile([128, 128], f16, tag="ident")
        make_identity(nc, ident[:])
        zeros = cp.tile([128, 256], f32, tag="zeros")
        nc.gpsimd.memset(zeros[:], 0.0)

        m10f = cp.tile([2, 256], f32, tag="m10f")

        with tc.tile_pool(name="pg", bufs=1, space="PSUM") as pg, \
             tc.tile_pool(name="pb", bufs=1, space="PSUM") as pb, \
             tc.tile_pool(name="pc", bufs=1, space="PSUM") as pc, \
             tc.tile_pool(name="ph", bufs=2, space="PSUM") as ph, \
             tc.tile_pool(name="pf", bufs=1, space="PSUM") as pf:

            # ---- P/S row sums via TS+accum, one op per (b, t) tile ------
            # Gc cols (t-major): [P_b0, S_b0, P_b1, S_b1] per t
            Gc = cp.tile([128, 8], f16, tag="Gc")
            for t, border in ((0, (0, 1)), (1, (1, 0))):
                for b in border:
                    wtile = wt4[:, 2 * b + t, :]
                    dS = sp.tile([128, 256], f16, tag="dump")
                    nc.vector.tensor_scalar(
                        dS[:], wtile, 0.0, None, Alu.add, op1=Alu.add,
                        accum_out=Gc[:, t * 4 + 2 * b + 1: t * 4 + 2 * b + 2],
                    )
                    dP = sp.tile([128, 256], f16, tag="dump")
                    nc.vector.tensor_scalar(
                        dP[:], wtile, 0.0, None, Alu.max, op1=Alu.add,
                        accum_out=Gc[:, t * 4 + 2 * b: t * 4 + 2 * b + 1],
                    )
                # transpose this t's 4 columns as soon as they are ready
                psT = pg.tile([4, 128], f16, tag="psT")
                nc.tensor.transpose(
                    psT[:], Gc[:, t * 4:(t + 1) * 4], ident[:]
                )
                nc.vector.tensor_copy(G2[0:4, t * 128:(t + 1) * 128], psT[:])

            # ---- base [128, 256] = F2.T @ G2 (K=6 fp16 matmul) ----------
            ps_base = pb.tile([128, 256], f32, tag="psbase")
            nc.tensor.matmul(ps_base[:], F2v, G2[:])

            # ---- message passing: 2 bias updates off colsums ------------
            cs1 = sp.tile([128, 1], f16, tag="cs1")
            d0 = sp.tile([128, 256], f16, tag="dump")
            nc.vector.tensor_scalar(
                d0[:], ps_base[:], 0.0, None, Alu.max, op1=Alu.add,
                accum_out=cs1[:],
            )
            ps_c1 = pc.tile([128, 1], f32, tag="psc")
            nc.tensor.matmul(ps_c1[:], w1hv, cs1[:], start=True, stop=False)
            nc.tensor.matmul(ps_c1[:], w1lv, cs1[:], start=False, stop=True)

            cs2 = sp.tile([128, 1], f16, tag="cs2")
            d1 = sp.tile([128, 256], f16, tag="dump")
            nc.vector.scalar_tensor_tensor(
                d1[:], ps_base[:], ps_c1[:, 0:1], zeros[:],
                Alu.add, Alu.max, accum_out=cs2[:],
            )
            ps_c2 = pc.tile([128, 1], f32, tag="psc")
            nc.tensor.matmul(ps_c2[:], w1hv, cs2[:], start=True, stop=False)
            nc.tensor.matmul(ps_c2[:], w1lv, cs2[:], start=False, stop=True)

            EMB = sp.tile([128, 256], f16, tag="EMB")
            nc.vector.tensor_scalar(
                EMB[:], ps_base[:], ps_c2[:, 0:1], 0.0, Alu.add, op1=Alu.max
            )
            # m10k = mask * 10 + kc; sits in the EMB->h2 vector gap
            nc.vector.tensor_scalar(
                m10f[:], mrowf[:], 10.0, wp[0:2, FW_KC:FW_KC + 1],
                Alu.mult, op1=Alu.add,
            )

            # ---- dueling heads (block-diagonal, both batches at once) ---
            ph1 = ph.tile([128, 256], f32, tag="pmat")
            nc.tensor.matmul(ph1[:], aw1v, EMB[:])
            h1 = sp.tile([128, 256], f16, tag="h1")
            nc.scalar.activation(h1[:], ph1[:], Act.Relu,
                                 bias=wp[:, FW_AB1:FW_AB1 + 1])

            phv = ph.tile([128, 256], f32, tag="pmat")
            nc.tensor.matmul(phv[:], vw1v, EMB[:])
            hvd = sp.tile([128, 256], f16, tag="hvd")
            hvcs = sp.tile([128, 1], f32, tag="hvcs")
            nc.scalar.activation(
                hvd[:], phv[:], Act.Relu, bias=wp[:, FW_VB1:FW_VB1 + 1],
                accum_out=hvcs[:],
            )
            pkb = pf.tile([2, 1], f32, tag="pkb")
            nc.tensor.matmul(pkb[:], vw2v, hvcs[:], start=True, stop=False)

            ph2 = ph.tile([64, 256], f32, tag="pmat")
            nc.tensor.matmul(ph2[:], aw2v, h1[:])
            h2 = sp.tile([64, 256], f16, tag="h2")
            h2cs = sp.tile([64, 1], f32, tag="h2cs")
            nc.vector.scalar_tensor_tensor(
                h2[:], ph2[:], wp[0:64, FW_AB2:FW_AB2 + 1], zeros[0:64, :],
                Alu.add, Alu.max, accum_out=h2cs[:],
            )
            pa = pf.tile([2, 256], f32, tag="pa")
            nc.tensor.matmul(pa[:], aw3v, h2[:])
            # pkb += (-a_w3bd)^T @ h2cs  ->  pkb = psv - ra
            nc.tensor.matmul(pkb[:], wp[0:64, FW_AW3F:FW_AW3F + 2], h2cs[:],
                             start=False, stop=True)

            # out = pa + [psv - ra] + (10*mask + kc)
            FIN = cp.tile([2, 256], f32, tag="FIN")
            nc.vector.scalar_tensor_tensor(
                FIN[:], pa[:], pkb[:, 0:1], m10f[:], Alu.add, Alu.add
            )
            nc.sync.dma_start(out[:, :], FIN[:])

    return nc


# ---------------------------------------------------------------------------
# GENERAL PATH (exact zeros in w): faithful per-batch adjacency matmuls
# (unchanged baseline implementation)
# ---------------------------------------------------------------------------
# wpack [64, NWP] f32 column layout (host-packed replicated params, one DMA).
WP_W1 = 0          # [64, 64] f32
WP_W2 = 64         # [64, 64] f32
WP_W3 = 128        # [64, 1] f32 column
WP_AB1 = 129       # [64, 1] f32
WP_VB1 = 130       # [64, 1] f32
WP_AB2 = 131       # [32, 1] f32 (padded)
WP_VW2 = 132       # [64, 1] f32
WP_AB3 = 133       # scalar at [0, 133]
WP_VB2 = 134       # scalar at [0, 134]
WP_AW1H = 135      # [64, 64] fp16 -> 32 f32 cols
WP_AW2H = 167      # [64, 32] fp16 -> 16 f32 cols
WP_AW3H = 183      # [32, 1] fp16 padded -> 1 f32 col
WP_VW1H = 184      # [64, 64] fp16 -> 32 f32 cols
WP_W0C = 216       # [64, 1] fp16 column (W0 transposed) -> 1 f32 col
WP_W2H = 217       # [64, 64] fp16 -> 32 f32 cols
NWP = 249


def _pack_weights(inputs: dict) -> np.ndarray:
    wp = np.zeros((64, NWP), dtype=np.float32)
    wp[:, WP_W1:WP_W1 + 64] = inputs["W1"]
    wp[:, WP_W2:WP_W2 + 64] = inputs["W2"]
    wp[:, WP_W3] = inputs["W3"][0]
    wp[:, WP_AB1] = inputs["a_b1"]
    wp[:, WP_VB1] = inputs["v_b1"]
    wp[:32, WP_AB2] = inputs["a_b2"]
    wp[:, WP_VW2] = inputs["v_w2"][:, 0]
    wp[0, WP_AB3] = inputs["a_b3"][0]
    wp[0, WP_VB2] = inputs["v_b2"][0]

    _place_f16(wp, 0, WP_AW1H, inputs["a_w1"])
    _place_f16(wp, 0, WP_AW2H, inputs["a_w2"])
    _place_f16(wp, 0, WP_AW3H, np.asarray(inputs["a_w3"])[:, 0:1])
    _place_f16(wp, 0, WP_VW1H, inputs["v_w1"])
    _place_f16(wp, 0, WP_W0C, np.asarray(inputs["W0"]).T)   # [64, 1]
    _place_f16(wp, 0, WP_W2H, inputs["W2"])
    return wp


def _build_general() -> bass.Bass:
    nc = bacc.Bacc(
        "TRN2", target_bir_lowering=False, debug=False, num_devices=NCORES
    )
    xs = nc.declare_dram_parameter("xs", [BPC, A + 2, A], f32, isOutput=False)
    wpd = nc.declare_dram_parameter("wpack", [64, NWP], f32, isOutput=False)
    out = nc.declare_dram_parameter("out", [BPC, A], f32, isOutput=True)

    with tile.TileContext(nc) as tc, ExitStack() as ctx:
        cp = ctx.enter_context(tc.tile_pool(name="const", bufs=1))
        sp = ctx.enter_context(tc.tile_pool(name="scratch", bufs=2))

        wp = cp.tile([64, NWP], f32, tag="wp")
        nc.sync.dma_start(wp[:], wpd[:])
        wt4 = cp.tile([128, 2 * BPC, A], f32, tag="wt4")
        for b in range(BPC):
            nc.scalar.dma_start(
                wt4[:, 2 * b: 2 * b + 2, :],
                xs[b, 2: A + 2, :].rearrange("(t p) j -> p t j", p=128),
            )
        csc = cp.tile([128, 2 * BPC], f32, tag="csc")
        for b in range(BPC):
            nc.gpsimd.dma_start(
                csc[:, 2 * b: 2 * b + 2],
                xs[b, 0, :].rearrange("(t p) -> p t", p=128),
            )
        mrow = cp.tile([1, BPC * A], f32, tag="mrow")
        nc.gpsimd.dma_start(
            mrow[:].rearrange("p (b a) -> p b a", b=BPC),
            xs[:, 1, :][None, :, :],
        )

        aw1h = wp[:, WP_AW1H:WP_AW1H + 32].bitcast(f16)
        aw2h = wp[:, WP_AW2H:WP_AW2H + 16].bitcast(f16)
        aw3h = wp[0:32, WP_AW3H:WP_AW3H + 1].bitcast(f16)[:, 0:1]
        vw1h = wp[:, WP_VW1H:WP_VW1H + 32].bitcast(f16)
        w0c = wp[:, WP_W0C:WP_W0C + 1].bitcast(f16)[:, 0:1]

        ident = cp.tile([128, 128], f16, tag="ident")
        make_identity(nc, ident[:])
        identf = cp.tile([128, 128], f32, tag="identf")
        make_identity(nc, identf[:])

        with tc.tile_pool(name="psetup", bufs=2, space="PSUM") as psetup:
            w2h = wp[:, WP_W2H:WP_W2H + 32].bitcast(f16)
            w3p = cp.tile([U, 1], f16, tag="w3p")
            nc.scalar.activation(w3p[:], wp[:, WP_W3:WP_W3 + 1], Act.Relu)
            w3n = cp.tile([U, 1], f16, tag="w3n")
            nc.scalar.activation(w3n[:], wp[:, WP_W3:WP_W3 + 1], Act.Relu,
                                 scale=-1.0)
            Fc = cp.tile([U, 3], f16, tag="Fc")
            nc.vector.tensor_copy(Fc[:, 0:1], w0c)
            pspc = psetup.tile([U, 1], f32, tag="pscol")
            nc.tensor.matmul(pspc[:], w2h, w3p[:])
            nc.scalar.mul(Fc[:, 1:2], pspc[:], INV_A)
            psnc = psetup.tile([U, 1], f32, tag="pscol")
            nc.tensor.matmul(psnc[:], w2h, w3n[:])
            nc.scalar.mul(Fc[:, 2:3], psnc[:], INV_A)
            psF = psetup.tile([3, U], f16, tag="psF")
            nc.tensor.transpose(psF[:], Fc[:], ident[0:U, 0:U])
            F = cp.tile([3, U], f16, tag="F")
            nc.vector.tensor_copy(F[:], psF[:])

        t256 = cp.tile([1, 1], f32, tag="t256")
        nc.gpsimd.tensor_scalar(
            t256[:], wp[0:1, WP_VB2:WP_VB2 + 1], float(A), None, Alu.mult
        )
        kc = cp.tile([1, 1], f32, tag="kc")
        nc.gpsimd.tensor_scalar(
            kc[:], wp[0:1, WP_AB3:WP_AB3 + 1], -float(A - 1), t256[:],
            Alu.mult, Alu.add,
        )

        m10 = cp.tile([1, BPC * A], f32, tag="m10")
        nc.scalar.mul(m10[:], mrow[:], 10.0)

        FIN = cp.tile([1, BPC * A], f32, tag="FIN")

        with tc.tile_pool(name="pmm", bufs=1, space="PSUM") as pmm, \
             tc.tile_pool(name="pbase", bufs=2, space="PSUM") as pbase, \
             tc.tile_pool(name="phead", bufs=2, space="PSUM") as phead:
            for b in range(BPC):
                Tb = sp.tile([128, 2], f32, tag="Tb")
                nc.vector.tensor_reduce(
                    Tb[:], wt4[:, 2 * b: 2 * b + 2, :], axis=AxX, op=Alu.add,
                    apply_absolute_value=True,
                )
                Sb = sp.tile([128, 2], f32, tag="Sb")
                nc.vector.tensor_reduce(
                    Sb[:], wt4[:, 2 * b: 2 * b + 2, :], axis=AxX, op=Alu.add
                )
                Sh = sp.tile([128, 2], f32, tag="Sh")
                nc.gpsimd.tensor_scalar(Sh[:], Sb[:], 0.5, None, Alu.mult)

                G = sp.tile([3, A], f16, tag="G")
                for t in range(2):
                    Cc = sp.tile([128, 3], f16, tag="Cc")
                    nc.gpsimd.tensor_copy(
                        Cc[:, 0:1], csc[:, 2 * b + t: 2 * b + t + 1]
                    )
                    nc.vector.scalar_tensor_tensor(
                        Cc[:, 1:2], Tb[:, t: t + 1], 0.5, Sh[:, t: t + 1],
                        Alu.mult, Alu.add,
                    )
                    nc.vector.scalar_tensor_tensor(
                        Cc[:, 2:3], Tb[:, t: t + 1], 0.5, Sh[:, t: t + 1],
                        Alu.mult, Alu.subtract,
                    )
                    tpc = pmm.tile([3, 128], f16, tag="tp1")
                    nc.tensor.transpose(tpc[:], Cc[:], ident[:])
                    nc.vector.tensor_copy(
                        G[:, t * 128: (t + 1) * 128], tpc[:]
                    )

                ps_base = pbase.tile([U, A], f32, tag="psbase")
                nc.tensor.matmul(ps_base[:], F[:], G[:])

                wt = wt4[:, 2 * b: 2 * b + 2, :]
                adjT = sp.tile([128, 2, A], f32, tag="adjT")
                for at in range(2):
                    for jt in range(2):
                        ptr = pmm.tile([128, 128], f32, tag="tp1")
                        nc.tensor.transpose(
                            ptr[:], wt[:, at, jt * 128: (jt + 1) * 128],
                            identf[:],
                        )
                        nc.vector.tensor_scalar(
                            adjT[:, jt, at * 128: (at + 1) * 128],
                            ptr[:], 0.0, None, Alu.not_equal,
                        )
                embT = sp.tile([U, A], f32, tag="embT")
                nc.vector.tensor_scalar(
                    embT[:], ps_base[:], 0.0, None, Alu.max
                )
                EMBb = None
                for it in range(2):
                    nat = sp.tile([128, 2, U], f32, tag="nat")
                    for ht in range(2):
                        pnat = pmm.tile([128, U], f32, tag="tp1")
                        nc.tensor.transpose(
                            pnat[:], embT[:, ht * 128: (ht + 1) * 128],
                            identf[0:U, 0:U],
                        )
                        nc.vector.tensor_copy(nat[:, ht, :], pnat[:])
                    ps_y = pmm.tile([U, A], f32, tag="tp1")
                    nc.tensor.matmul(ps_y[:], nat[:, 0, :], adjT[:, 0, :],
                                     start=True, stop=False)
                    nc.tensor.matmul(ps_y[:], nat[:, 1, :], adjT[:, 1, :],
                                     start=False, stop=True)
                    ysb = sp.tile([U, A], f32, tag="ysb")
                    nc.vector.tensor_scalar(ysb[:], ps_y[:], INV_A, None,
                                            Alu.mult)
                    ps_it = pbase.tile([U, A], f32, tag="psbase")
                    nc.tensor.matmul(ps_it[:], F[:], G[:],
                                     start=True, stop=False)
                    nc.tensor.matmul(ps_it[:], wp[:, WP_W1:WP_W1 + 64],
                                     ysb[:], start=False, stop=True)
                    if it == 0:
                        embT = sp.tile([U, A], f32, tag="embT")
                        nc.vector.tensor_scalar(
                            embT[:], ps_it[:], 0.0, None, Alu.max
                        )
                    else:
                        EMBb = sp.tile([U, A], f16, tag="EMBb")
                        nc.vector.tensor_scalar(
                            EMBb[:], ps_it[:], 0.0, None, Alu.max
                        )

                sl = slice(b * A, (b + 1) * A)
                ph1 = phead.tile([HID, A], f32, tag="pmat")
                nc.tensor.matmul(ph1[:], aw1h, EMBb[:])
                h1 = sp.tile([HID, A], f16, tag="h1")
                nc.scalar.activation(h1[:], ph1[:], Act.Relu,
                                     bias=wp[:, WP_AB1:WP_AB1 + 1])
                ph2 = phead.tile([HID // 2, A], f32, tag="pmat")
                nc.tensor.matmul(ph2[:], aw2h, h1[:])
                h2 = sp.tile([HID // 2, A], f16, tag="h2")
                nc.vector.tensor_scalar(
                    h2[:], ph2[:], wp[0:32, WP_AB2:WP_AB2 + 1], 0.0,
                    Alu.add, op1=Alu.max,
                )
                pa = phead.tile([1, A], f32, tag="pa")
                nc.tensor.matmul(pa[:], aw3h, h2[:])

                phv = phead.tile([HID, A], f32, tag="pmat")
                nc.tensor.matmul(phv[:], vw1h, EMBb[:])
                hv = sp.tile([HID, A], f32, tag="hv")
                hv_cs = sp.tile([U, 1], f32, tag="hv_cs")
                nc.scalar.activation(hv[:], phv[:], Act.Relu,
                                     bias=wp[:, WP_VB1:WP_VB1 + 1],
                                     accum_out=hv_cs[:])
                psv = phead.tile([1, 1], f32, tag="pa")
                nc.tensor.matmul(psv[:], hv_cs[:], wp[:, WP_VW2:WP_VW2 + 1])

                ra = sp.tile([1, 1], f32, tag="ra")
                nc.vector.tensor_reduce(ra[:], pa[:], axis=AxX, op=Alu.add)
                Kb = sp.tile([1, 1], f32, tag="Kb")
                nc.vector.tensor_scalar(
                    Kb[:], psv[:], ra[:], kc[:], Alu.subtract, op1=Alu.add
                )
                nc.vector.scalar_tensor_tensor(
                    FIN[:, sl], pa[:], Kb[:], m10[:, sl], Alu.add, Alu.add
                )
                if b == 0:
                    nc.sync.dma_start(out[b, :][None, :], FIN[:, sl])
                else:
                    nc.scalar.dma_start(out[b, :][None, :], FIN[:, sl])

    return nc


_NC_CACHE: dict[bool, bass.Bass] = {}


def _get_nc(fast: bool) -> bass.Bass:
    if fast not in _NC_CACHE:
        nc = _build_fast() if fast else _build_general()
        nc.finalize()
        _NC_CACHE[fast] = nc
    return _NC_CACHE[fast]


def _make_in_maps(inputs: dict, fast: bool) -> list[dict]:
    x = np.ascontiguousarray(np.asarray(inputs["x"], dtype=np.float32))
    wd = {k: np.asarray(inputs[k], dtype=np.float32) for k in WEIGHT_NAMES}
    wname = "wp2" if fast else "wpack"
    wpk = _pack_weights_fast(wd) if fast else _pack_weights(wd)
    in_maps = []
    for c in range(NCORES):
        in_maps.append({
            "xs": np.ascontiguousarray(x[c * BPC: (c + 1) * BPC]),
            wname: wpk,
        })
    return in_maps


def run(inputs: dict, trace: bool = False, tmpdir: str | None = None):
    """Returns (output [B, A] f32, BassKernelResults)."""
    x = np.asarray(inputs["x"])
    fast = bool((x[:, 2:, :] != 0.0).all())
    nc = _get_nc(fast)
    res = run_bass_kernel_spmd(
        nc, _make_in_maps(inputs, fast), list(range(NCORES)),
        trace=trace, tmpdir=tmpdir,
    )
    out = np.concatenate([res.results[i]["out"] for i in range(NCORES)], axis=0)
    return out, res


def kernel(**inputs) -> np.ndarray:
    out, _ = run(inputs)
    return out
